# revision 1
# baseline (speedup 1.0000x reference)
"""Trainium2 Bass kernel for nn_MultiHeadAttention (B=4, T=2048, D=1024,
H=16, d_k=64) on 8 NeuronCores.

Sharding: tensor-parallel over heads — core c computes heads {2c, 2c+1} for
ALL batches (W_q/W_k/W_v column-sharded, W_o row-sharded). The final
all-reduce of the output projection is replaced by a host-side sum of the 8
partial outputs (each written transposed, [D, T]).

Design (evolved across nine profiled hardware iterations):
  - K/V are projected (and their x DMA'd) only for the ceil(vl/128) Tk
    tiles that attention actually reads — 24 of 64 tiles for this input
    distribution (saves ~47us PE + ~21MB HBM per core vs projecting all).
  - scores^T layout (Tk on partitions, Tq free): the two heads' QK^T
    matmuls are K=64 row-tile pairs sharing the PE array; the padding mask
    is premixed into the scores PSUM by a K=1 matmul (mask-row x 8 as lhsT,
    ones as rhs), so every exp() is bias-free and two Tk tiles merge into
    one [128, 2x512] ACT instruction (the 352-cycle ACT overhead was 40% of
    exp cost at single-tile granularity; exp is the critical engine).
  - softmax denominator rides as a ones-column folded into the P@V matmul
    (lhsT = [V_h | 1]); the un-normalized outputs + den rows are staged with
    one [65, 512] DVE copy; reciprocals are batched (bf16 is plenty);
    1/den is broadcast across partitions with a K=1 matmul and applied by
    DVE multiplies.
  - output projection keeps W_o stationary (LDWEIGHTS amortized over 4
    N=512 streams) and writes the partial transposed, [D, T].
  - emission is software-pipelined at instruction granularity: per-engine
    queues are in-order, so projection / normalization / out-projection
    work is pumped in small units between attention iterations, with
    out-projection PSUM->SBUF copies steered to ACT only in exp-sparse
    windows (they otherwise head-block the exp server). Slot order puts
    the second-largest batch first (small startup bubble) and the largest
    second, whose long exp window absorbs the remaining background work.
  - trn2 encodes at most one semaphore wait per instruction; a post-pass
    splits any multi-wait instruction Tile emits into single-wait
    InstEventSemaphore ops (walrus rejects them otherwise).
"""
import os
import sys
from collections import deque

for _p in ("/opt/trn_rl_repo", "/root/.axon_site/_ro/trn_rl_repo"):
    if os.path.isdir(_p) and _p not in sys.path:
        sys.path.append(_p)

import numpy as np
import ml_dtypes

import concourse.bass as bass
import concourse.mybir as mybir
import concourse.tile as tile
from concourse.bass import ts
from concourse.bass_utils import run_bass_kernel_spmd

D = 1024
T = 2048
H = 16
DK = 64
P = 128
KC = D // P          # 8 contraction chunks for the projections
TC = T // P          # 16 token tiles of 128
NT = T // 512        # 4 Tq chunks of 512
NCORES = 8
MASK_NEG = -30000.0

F32 = mybir.dt.float32
F32R = mybir.dt.float32r
BF16 = mybir.dt.bfloat16
AF = mybir.ActivationFunctionType
BF16_NP = ml_dtypes.bfloat16


def _split_multi_waits(nc):
    """trn2 instructions encode at most one sync wait; split the rest into
    standalone single-wait event-semaphore ops."""
    n_split = 0
    for f in nc.m.functions:
        for blk in f.blocks:
            insts = blk.instructions
            out = []
            changed = False
            for inst in insts:
                si = inst.sync_info
                if si is not None and len(si.on_wait) > 1:
                    waits = list(si.on_wait)
                    for k, wt in enumerate(waits[:-1]):
                        ev = mybir.InstEventSemaphore(
                            name=f"{inst.name}_wsplit{k}",
                            engine=inst.engine,
                            ins=[],
                            outs=[],
                            bass_nofuse=True,
                            sync_info=mybir.SyncInfo(on_wait=[wt], on_update=[]),
                        )
                        out.append(ev)
                        n_split += 1
                    inst.sync_info = mybir.SyncInfo(
                        on_wait=[waits[-1]], on_update=si.on_update
                    )
                    changed = True
                out.append(inst)
            if changed:
                blk.instructions = out
    return n_split


def build_nc(NB, J_list, dt_x):
    """Build the SPMD program.

    NB     : number of batch slots handled per core
    J_list : per batch slot, number of 128-row Tk tiles of attention
    dt_x   : dtype of x/weights/intermediates
    """
    nc = bass.Bass()

    # partition-major tile layout: per partition, each 128-token tile is a
    # contiguous [KC, 128] run (2 KB bf16)
    xq_d = [nc.declare_dram_parameter(f"xq{s}", [P, TC, KC, P], dt_x,
                                      isOutput=False) for s in range(NB)]
    xk_d = [nc.declare_dram_parameter(f"xk{s}", [P, J_list[s], KC, P], dt_x,
                                      isOutput=False) for s in range(NB)]
    xv_d = [nc.declare_dram_parameter(f"xv{s}", [P, J_list[s], KC, P], dt_x,
                                      isOutput=False) for s in range(NB)]
    wq_d = nc.declare_dram_parameter("wq", [P, KC, P], dt_x, isOutput=False)
    wk_d = nc.declare_dram_parameter("wk", [P, KC, P], dt_x, isOutput=False)
    wv_d = nc.declare_dram_parameter("wv", [P, KC, P], dt_x, isOutput=False)
    wo_d = nc.declare_dram_parameter("wo", [P, KC, P], dt_x, isOutput=False)
    bq_d = nc.declare_dram_parameter("bq", [P, 1], F32, isOutput=False)
    bk_d = nc.declare_dram_parameter("bk", [P, 1], F32, isOutput=False)
    bv_d = nc.declare_dram_parameter("bv", [1, P], dt_x, isOutput=False)
    mb_d = [nc.declare_dram_parameter(f"mb{s}", [1, P], BF16, isOutput=False)
            for s in range(NB)]
    onesb_d = nc.declare_dram_parameter("onesb", [1, DK], BF16,
                                        isOutput=False)
    o_d = [nc.declare_dram_parameter(f"o{s}", [D, T], BF16, isOutput=True)
           for s in range(NB)]

    with tile.TileContext(nc) as tc:
        with (
            tc.tile_pool(name="pers", bufs=1) as pers,
            tc.tile_pool(name="stream", bufs=3) as stream,
            tc.tile_pool(name="attn", bufs=4) as attn_pool,
            tc.tile_pool(name="small", bufs=6) as small,
            tc.tile_pool(name="outp", bufs=8) as outp,
            tc.tile_pool(name="ps_qk", bufs=2, space="PSUM") as ps_qk,
            tc.tile_pool(name="ps_pv", bufs=2, space="PSUM") as ps_pv,
            tc.tile_pool(name="ps_bg", bufs=2, space="PSUM") as ps_bg,
        ):
            # ---- persistent tensors -------------------------------------
            wq = pers.tile([P, KC, P], dt_x, name="wq")
            wk = pers.tile([P, KC, P], dt_x, name="wk")
            wv = pers.tile([P, KC, P], dt_x, name="wv")
            wo = pers.tile([P, KC, P], dt_x, name="wo")
            bq = pers.tile([P, 1], F32, name="bq")
            bk = pers.tile([P, 1], F32, name="bk")
            bv = pers.tile([1, P], dt_x, name="bv")
            nc.sync.dma_start(wq[:], wq_d[:])
            nc.sync.dma_start(wk[:], wk_d[:])
            nc.sync.dma_start(wv[:], wv_d[:])
            nc.sync.dma_start(wo[:], wo_d[:])
            nc.sync.dma_start(bq[:], bq_d[:])
            nc.sync.dma_start(bk[:], bk_d[:])
            nc.sync.dma_start(bv[:], bv_d[:])
            mb = []
            for s in range(NB):
                t = pers.tile([1, P], BF16, name=f"mb{s}")
                nc.sync.dma_start(t[:], mb_d[s][:])
                mb.append(t)
            ones_r = pers.tile([1, 512], BF16, name="ones_r")
            nc.vector.memset(ones_r[:], 1.0)

            ones_t = pers.tile([1, P], dt_x, name="ones_t")   # V-bias fold lhsT
            nc.vector.memset(ones_t[:], 1.0)
            ones_b = pers.tile([1, DK], BF16, name="ones_b")  # 1/den bcast lhsT
            nc.sync.dma_start(ones_b[:], onesb_d[:])

            QT = [pers.tile([P, T], dt_x, name=f"QT{s}") for s in range(NB)]
            KT = [pers.tile([P, J_list[s] * P], dt_x, name=f"KT{s}")
                  for s in range(NB)]
            # V with a ones column folded in at free index 64 of each head
            V = [pers.tile([P, J_list[s], 2, DK + 1], dt_x, name=f"V{s}")
                 for s in range(NB)]
            for s in range(NB):
                nc.vector.memset(V[s][:, :, :, DK], 1.0)

            AO = [pers.tile([P, T], dt_x, name=f"AO{s}") for s in range(NB)]
            NR = 2 * NT  # unnormalized-output rows per slot (tq, head)
            uo = [pers.tile([DK + 1, NR, 512], BF16, name=f"uo{s}")
                  for s in range(NB)]
            dens = [pers.tile([NR, 512], BF16, name=f"dens{s}")
                    for s in range(NB)]
            recs = [pers.tile([NR, 512], BF16, name=f"rec{s}")
                    for s in range(NB)]

            # ---- emission-unit generators -------------------------------
            def proj_gen(s):
                J = J_list[s]
                # K projection: weight-stationary per kc over <=4-tile chunks
                ngr = -(-J // 4)
                for g in range(ngr):
                    t0 = 4 * g
                    nt_ = min(4, J - t0)
                    xkw = stream.tile([P, 4, KC, P], dt_x, tag="xk_w")
                    nc.sync.dma_start(xkw[:, 0:nt_], xk_d[s][:, t0:t0 + nt_])
                    ps_k = ps_bg.tile([P, 512], F32, tag="bg")
                    for kc in range(KC):
                        nc.tensor.matmul(ps_k[:, 0:nt_ * P], wk[:, kc, :],
                                         xkw[:, 0:nt_, kc, :],
                                         start=(kc == 0), stop=(kc == KC - 1))
                        if kc == 3:
                            yield
                    nc.vector.tensor_scalar_add(
                        KT[s][:, t0 * P:(t0 + nt_) * P],
                        ps_k[:, 0:nt_ * P], bk[:, 0:1])
                    yield
                # V projection: x-tile stationary, wv moving (N=128)
                for g in range(ngr):
                    t0 = 4 * g
                    nt_ = min(4, J - t0)
                    xvw = stream.tile([P, 4, KC, P], dt_x, tag="xv_w")
                    nc.sync.dma_start(xvw[:, 0:nt_], xv_d[s][:, t0:t0 + nt_])
                    ps_v = ps_bg.tile([P, 512], F32, tag="bg")
                    for i in range(nt_):
                        reg = ps_v[:, ts(i, P)]
                        for kc in range(KC):
                            nc.tensor.matmul(reg, xvw[:, i, kc, :],
                                             wv[:, kc, :],
                                             start=(kc == 0), stop=False)
                        nc.tensor.matmul(reg, ones_t[0:1, :], bv[0:1, :],
                                         start=False, stop=True)
                        yield
                    for i in range(nt_):
                        nc.vector.tensor_copy(
                            V[s][:, t0 + i, :, 0:DK],
                            ps_v[:, ts(i, P)].rearrange("p (h d) -> p h d",
                                                        d=DK))
                    yield
                # Q projection: weight-stationary per kc over 4-tile chunks
                for g in range(NT):
                    xqw = stream.tile([P, 4, KC, P], dt_x, tag="xq_w")
                    nc.sync.dma_start(xqw[:], xq_d[s][:, 4 * g:4 * g + 4])
                    ps_q = ps_bg.tile([P, 512], F32, tag="bg")
                    for kc in range(KC):
                        nc.tensor.matmul(ps_q[:], wq[:, kc, :],
                                         xqw[:, :, kc, :],
                                         start=(kc == 0), stop=(kc == KC - 1))
                        if kc == 3:
                            yield
                    nc.vector.tensor_scalar_add(QT[s][:, ts(g, 512)],
                                                ps_q[:], bq[:, 0:1])
                    yield

            def attn_emit(s, pump):
                J = J_list[s]
                # j-tile pair schedule (the last tile's mask is premixed
                # into its psum, so it merges like any interior tile)
                items = [tuple(range(j, min(j + 2, J)))
                         for j in range(0, J, 2)]
                for tq in range(NT):
                    ps_os = [ps_pv.tile([P, 512], F32, tag="pv",
                                        name=f"pv{h}")
                             for h in range(2)]
                    # unit = (item, head): emit QK+exp for unit u, then the
                    # PV of unit u-1, so the PE never heads-of-line-waits on
                    # an exp (per-engine queues are in-order)
                    pv_pending = deque()
                    for it in items:
                        for h in range(2):
                            pss = ps_qk.tile([P, 2, 512], F32, tag="qk")
                            for k, j_ in enumerate(it):
                                masked = j_ == J - 1
                                nc.tensor.matmul(
                                    pss[:, k, :],
                                    KT[s][ts(h, DK), ts(j_, P)],
                                    QT[s][ts(h, DK), ts(tq, 512)],
                                    start=True, stop=not masked,
                                    tile_position=(h * DK, 0))
                                if masked:
                                    nc.tensor.matmul(
                                        pss[:, k, :], mb[s][0:1, :],
                                        ones_r[0:1, :],
                                        start=False, stop=True)
                            at = attn_pool.tile([P, 2, 512], dt_x, tag="at")
                            if len(it) == 2:
                                nc.scalar.activation(at[:, :, :], pss[:],
                                                     AF.Exp, scale=0.125)
                            else:
                                nc.scalar.activation(at[:, 0, :],
                                                     pss[:, 0, :], AF.Exp,
                                                     scale=0.125)
                            for k, j_ in enumerate(it):
                                pv_pending.append((j_, h, at, k))
                            while len(pv_pending) > 2:
                                _emit_pv(s, tq, ps_os, pv_pending.popleft())
                            pump()
                    while pv_pending:
                        _emit_pv(s, tq, ps_os, pv_pending.popleft())
                    for h in range(2):
                        r = tq * 2 + h
                        # den row rides along at partition DK (bf16 is
                        # plenty for the softmax denominator)
                        nc.vector.tensor_copy(uo[s][:, r, :],
                                              ps_os[h][0:DK + 1, :])
                        nc.sync.dma_start(dens[s][r:r + 1, :],
                                          uo[s][DK:DK + 1, r, :])
                    pump()

            def _emit_pv(s, tq, ps_os, unit):
                J = J_list[s]
                j_, h, at, k = unit
                nc.tensor.matmul(ps_os[h][0:DK + 1, :],
                                 V[s][:, j_, h, :], at[:, k, :],
                                 start=(j_ == 0), stop=(j_ == J - 1))

            def norm_gen(s):
                # batched normalization (bf16 denominators are plenty)
                with nc.allow_low_precision(reason="bf16 1/den is plenty"):
                    nc.vector.reciprocal(recs[s][:], dens[s][:])
                yield
                for r in range(NR):
                    # stage rec row at partition 0 for the K=1 bcast matmul
                    rst = small.tile([1, 512], BF16, tag="rst")
                    nc.sync.dma_start(rst[:], recs[s][r:r + 1, :])
                    ps_b = ps_bg.tile([P, 512], F32, tag="bg")
                    nc.tensor.matmul(ps_b[0:DK, :], ones_b[0:1, :],
                                     rst[0:1, :], start=True, stop=True)
                    h = r % 2
                    tq = r // 2
                    nc.vector.tensor_mul(
                        out=AO[s][ts(h, DK), ts(tq, 512)],
                        in0=ps_b[0:DK, :], in1=uo[s][0:DK, r, :])
                    yield

            def outproj_gen(s, use_act):
                # output projection, pumped into later attention windows;
                # copies go to ACT only where the exp stream is sparse
                unit = 0
                for dt_i in range(KC):
                    for n in range(NT):
                        ps_op = ps_bg.tile([P, 512], F32, tag="bg")
                        nc.tensor.matmul(ps_op[:], wo[:, dt_i, :],
                                         AO[s][:, ts(n, 512)],
                                         start=True, stop=True)
                        ot = outp.tile([P, 512], BF16, tag="ot")
                        if use_act and unit % 2 == 0:
                            nc.scalar.activation(ot[:], ps_op[:],
                                                 AF.Identity)
                        else:
                            nc.vector.tensor_copy(ot[:], ps_op[:])
                        nc.sync.dma_start(o_d[s][ts(dt_i, P), ts(n, 512)],
                                          ot[:])
                        unit += 1
                        yield

            # ---- software-pipelined emission ----------------------------
            pending = deque()

            def pump(n=2):
                k = 0
                while pending and k < n:
                    try:
                        next(pending[0][1])
                        k += 1
                    except StopIteration:
                        pending.popleft()

            def drain_proj():
                while pending and pending[0][0] == "proj":
                    try:
                        next(pending[0][1])
                    except StopIteration:
                        pending.popleft()

            for u in proj_gen(0):
                pass
            for s in range(NB):
                if s + 1 < NB:
                    pending.append(("proj", proj_gen(s + 1)))
                if s >= 1:
                    pending.append(("norm", norm_gen(s - 1)))
                    # ACT helps with copies only in exp-sparse windows
                    pending.append(("op", outproj_gen(s - 1,
                                                     J_list[s] < 8)))
                attn_emit(s, pump)
                drain_proj()
            pending.append(("norm", norm_gen(NB - 1)))
            pending.append(("op", outproj_gen(NB - 1, True)))
            while pending:
                pump(1000)

    _split_multi_waits(nc)
    return nc


_CACHE = {}


def _get_nc(NB, J_list, dt_x):
    key = (NB, tuple(J_list), str(dt_x))
    if key not in _CACHE:
        _CACHE[key] = build_nc(NB, J_list, dt_x)
    return _CACHE[key]


def _xt(x, dt_np, ntiles=TC):
    """[T, D] -> [P, ntiles, KC, 128] partition-major tile layout."""
    xt = x.T.reshape(KC, P, TC, P).transpose(1, 2, 0, 3)[:, :ntiles]
    return np.ascontiguousarray(xt).astype(dt_np)


def kernel(**inputs):
    query = np.asarray(inputs["query"], dtype=np.float32)
    key = np.asarray(inputs["key"], dtype=np.float32)
    value = np.asarray(inputs["value"], dtype=np.float32)
    vl = np.asarray(inputs["valid_length"]).astype(np.int64)
    W_q = np.asarray(inputs["W_q"], dtype=np.float32)
    b_q = np.asarray(inputs["b_q"], dtype=np.float32)
    W_k = np.asarray(inputs["W_k"], dtype=np.float32)
    b_k = np.asarray(inputs["b_k"], dtype=np.float32)
    W_v = np.asarray(inputs["W_v"], dtype=np.float32)
    b_v = np.asarray(inputs["b_v"], dtype=np.float32)
    W_o = np.asarray(inputs["W_o"], dtype=np.float32)
    b_o = np.asarray(inputs["b_o"], dtype=np.float32)

    B = query.shape[0]
    NB = B
    CPB = (H // NCORES) * DK       # 2 heads per core -> 128 cols
    dt_x = BF16
    dt_np = BF16_NP

    # slot s handles batch order[s]: second-largest first (small-ish
    # startup bubble), then the largest (its long exp window absorbs the
    # later projections and copies), then descending
    Jv = np.where(vl == 0, TC * P, np.minimum(vl, TC * P))
    order = list(np.argsort(-Jv, kind="stable"))
    order[0], order[1] = order[1], order[0]
    J_list = []
    for s in range(NB):
        v = int(vl[order[s]])
        J_list.append(TC if v == 0 else max(1, -(-v // P)))

    nc = _get_nc(NB, J_list, dt_x)

    # host-side shard prep
    xq_np, xk_np, xv_np, mb_np = [], [], [], []
    for s in range(NB):
        b = int(order[s])
        v = int(vl[b])
        J = J_list[s]
        q_b = query[b] if v != 0 else np.zeros_like(query[b])
        xq_np.append(_xt(q_b, dt_np))
        xk_np.append(_xt(key[b], dt_np, J))
        xv_np.append(_xt(value[b], dt_np, J))
        rows = np.arange(P) + (J - 1) * P
        if v == 0:
            m = np.zeros((1, P), np.float32)
        else:
            m = np.where(rows < v, 0.0, 8.0 * MASK_NEG)[None, :]
        mb_np.append(np.ascontiguousarray(m).astype(BF16_NP))

    in_maps = []
    for c in range(NCORES):
        c0 = c * CPB
        cols = slice(c0, c0 + CPB)
        im = {
            "wq": np.ascontiguousarray(
                W_q.reshape(KC, P, H * DK).transpose(1, 0, 2)[:, :, cols]
            ).astype(dt_np),
            "wk": np.ascontiguousarray(
                W_k.reshape(KC, P, H * DK).transpose(1, 0, 2)[:, :, cols]
            ).astype(dt_np),
            "wv": np.ascontiguousarray(
                W_v.reshape(KC, P, H * DK).transpose(1, 0, 2)[:, :, cols]
            ).astype(dt_np),
            "wo": np.ascontiguousarray(
                W_o[cols].reshape(P, KC, P)).astype(dt_np),
            "bq": np.ascontiguousarray(b_q[cols][:, None]).astype(np.float32),
            "bk": np.ascontiguousarray(b_k[cols][:, None]).astype(np.float32),
            "bv": np.ascontiguousarray(b_v[cols][None, :]).astype(dt_np),
        }
        im["onesb"] = np.ones((1, DK), BF16_NP)
        for s in range(NB):
            im[f"xq{s}"] = xq_np[s]
            im[f"xk{s}"] = xk_np[s]
            im[f"xv{s}"] = xv_np[s]
            im[f"mb{s}"] = mb_np[s]
        in_maps.append(im)

    res = run_bass_kernel_spmd(nc, in_maps, list(range(NCORES)))

    out = np.zeros((B, T, D), np.float32)
    for s in range(NB):
        b = int(order[s])
        acc = np.zeros((D, T), np.float32)
        for c in range(NCORES):
            acc += np.asarray(res.results[c][f"o{s}"]).astype(np.float32)
        out[b] = acc.T + b_o[None, :]
    return out



# revision 11
# speedup vs baseline: 1.1802x; 1.1802x over previous
"""Trainium2 Bass kernel for nn_MultiHeadAttention (B=4, T=2048, D=1024,
H=16, d_k=64) on 8 NeuronCores.

Sharding: tensor-parallel over heads — core c computes heads {2c, 2c+1} for
ALL batches (W_q/W_k/W_v column-sharded, W_o row-sharded). The final
all-reduce of the output projection is replaced by a host-side sum of the 8
partial outputs (each written as [KC, 128, T]).

Design (v10 — rebuilt from the v9 trace):
  - K/V are projected (and their x DMA'd) only for the ceil(vl/128) Tk
    tiles that attention actually reads.
  - scores^T layout (Tk on partitions, Tq free): the two heads' QK^T
    matmuls are K=64 row-tile pairs sharing the PE array concurrently via
    tile_position; two Tk tiles merge into one [128, 2x512] ACT exp.
  - NO score masking: invalid key rows (>= vl, last tile only) are instead
    zeroed in V (both the value columns and the folded ones-column), so
    their exp'd scores contribute nothing to numerator or denominator.
    Removes 32 K=1 premix matmuls + the mask DMAs.
  - softmax denominator rides as a ones-column folded into the P@V matmul
    (lhsT = [V_h | 1]); un-normalized outputs + den rows staged per tq;
    dens gathered with ONE DMA per slot, reciprocal'd in 4 pumpable DVE
    chunks, re-staged to partition 0 with ONE DMA, broadcast by K=1
    matmuls, applied by DVE multiplies.
  - slot order: descending J with the LARGEST batch LAST — its long exp
    window (ACT-bound, PE slack) absorbs the out-projections and
    normalizations of all earlier slots, which previously formed a
    ~150us copy-bound serial tail with a cold (HAM-throttled) PE.
  - out-projection stages 4 n-chunks into a [128, 2048] SBUF tile and
    writes ONE DMA per (slot, dt): 32 output DMAs instead of 128 (the
    Sync trigger queue at ~707ns/DMA was 70%+ busy in the tail).
  - PSUM->SBUF copies are steered: ACT (Identity) in exp-sparse windows,
    DVE in the exp-saturated big window, alternating in drain/tail.
  - emission is software-pipelined at instruction granularity (in-order
    per-engine queues): proj/norm/outproj generators are pumped between
    attention units; the pre-big-slot drain interleaves proj with
    norm/outproj so DVE/ACT stay fed while the PE runs projections.
  - trn2 encodes at most one semaphore wait per instruction; a post-pass
    splits any multi-wait instruction Tile emits.
"""
import os
import sys
from collections import deque

for _p in ("/opt/trn_rl_repo", "/root/.axon_site/_ro/trn_rl_repo"):
    if os.path.isdir(_p) and _p not in sys.path:
        sys.path.append(_p)

import numpy as np
import ml_dtypes

import concourse.bass as bass
import concourse.mybir as mybir
import concourse.tile as tile
from concourse.bass import ts
from concourse.bass_utils import run_bass_kernel_spmd

D = 1024
T = 2048
H = 16
DK = 64
P = 128
KC = D // P          # 8 contraction chunks for the projections
TC = T // P          # 16 token tiles of 128
NT = T // 512        # 4 Tq chunks of 512
NCORES = 8

F32 = mybir.dt.float32
BF16 = mybir.dt.bfloat16
AF = mybir.ActivationFunctionType
BF16_NP = ml_dtypes.bfloat16


def _split_multi_waits(nc):
    """trn2 instructions encode at most one sync wait; split the rest into
    standalone single-wait event-semaphore ops."""
    n_split = 0
    for f in nc.m.functions:
        for blk in f.blocks:
            insts = blk.instructions
            out = []
            changed = False
            for inst in insts:
                si = inst.sync_info
                if si is not None and len(si.on_wait) > 1:
                    waits = list(si.on_wait)
                    for k, wt in enumerate(waits[:-1]):
                        ev = mybir.InstEventSemaphore(
                            name=f"{inst.name}_wsplit{k}",
                            engine=inst.engine,
                            ins=[],
                            outs=[],
                            bass_nofuse=True,
                            sync_info=mybir.SyncInfo(on_wait=[wt], on_update=[]),
                        )
                        out.append(ev)
                        n_split += 1
                    inst.sync_info = mybir.SyncInfo(
                        on_wait=[waits[-1]], on_update=si.on_update
                    )
                    changed = True
                out.append(inst)
            if changed:
                blk.instructions = out
    return n_split


def build_nc(NB, J_list, P0_list, dt_x):
    """Build the SPMD program.

    NB      : number of batch slots handled per core
    J_list  : per batch slot, number of 128-row Tk tiles of attention
    P0_list : per batch slot, valid rows in the LAST Tk tile (1..128)
    dt_x    : dtype of x/weights/intermediates
    """
    nc = bass.Bass()

    # partition-major tile layout: per partition, each 128-token tile is a
    # contiguous [KC, 128] run (2 KB bf16)
    xq_d = [nc.declare_dram_parameter(f"xq{s}", [P, TC, KC, P], dt_x,
                                      isOutput=False) for s in range(NB)]
    xk_d = [nc.declare_dram_parameter(f"xk{s}", [P, J_list[s], KC, P], dt_x,
                                      isOutput=False) for s in range(NB)]
    xv_d = [nc.declare_dram_parameter(f"xv{s}", [P, J_list[s], KC, P], dt_x,
                                      isOutput=False) for s in range(NB)]
    wq_d = nc.declare_dram_parameter("wq", [P, KC, P], dt_x, isOutput=False)
    wk_d = nc.declare_dram_parameter("wk", [P, KC, P], dt_x, isOutput=False)
    wv_d = nc.declare_dram_parameter("wv", [P, KC, P], dt_x, isOutput=False)
    wo_d = nc.declare_dram_parameter("wo", [P, KC, P], dt_x, isOutput=False)
    bq_d = nc.declare_dram_parameter("bq", [P, 1], F32, isOutput=False)
    bk_d = nc.declare_dram_parameter("bk", [P, 1], F32, isOutput=False)
    bv_d = nc.declare_dram_parameter("bv", [1, P], dt_x, isOutput=False)
    onesb_d = nc.declare_dram_parameter("onesb", [1, DK], BF16,
                                        isOutput=False)
    o_d = [nc.declare_dram_parameter(f"o{s}", [KC, P, T], BF16, isOutput=True)
           for s in range(NB)]

    with tile.TileContext(nc) as tc:
        with (
            tc.tile_pool(name="pers", bufs=1) as pers,
            tc.tile_pool(name="stream", bufs=3) as stream,
            tc.tile_pool(name="attn", bufs=4) as attn_pool,
            tc.tile_pool(name="small", bufs=2) as small,
            tc.tile_pool(name="outp", bufs=3) as outp,
            tc.tile_pool(name="ps_qk", bufs=2, space="PSUM") as ps_qk,
            tc.tile_pool(name="ps_pv", bufs=2, space="PSUM") as ps_pv,
            tc.tile_pool(name="ps_bg", bufs=2, space="PSUM") as ps_bg,
        ):
            # ---- persistent tensors -------------------------------------
            wq = pers.tile([P, KC, P], dt_x, name="wq")
            wk = pers.tile([P, KC, P], dt_x, name="wk")
            wv = pers.tile([P, KC, P], dt_x, name="wv")
            wo = pers.tile([P, KC, P], dt_x, name="wo")
            bq = pers.tile([P, 1], F32, name="bq")
            bk = pers.tile([P, 1], F32, name="bk")
            bv = pers.tile([1, P], dt_x, name="bv")
            # front-load only what the first K/V projections need
            nc.sync.dma_start(wk[:], wk_d[:])
            nc.sync.dma_start(wv[:], wv_d[:])
            nc.sync.dma_start(bk[:], bk_d[:])
            nc.sync.dma_start(bv[:], bv_d[:])

            ones_t = pers.tile([1, P], dt_x, name="ones_t")   # V-bias fold lhsT
            nc.vector.memset(ones_t[:], 1.0)
            ones_b = pers.tile([1, DK], BF16, name="ones_b")  # 1/den bcast lhsT

            QT = [pers.tile([P, T], dt_x, name=f"QT{s}") for s in range(NB)]
            KT = [pers.tile([P, J_list[s] * P], dt_x, name=f"KT{s}")
                  for s in range(NB)]
            # V with a ones column folded in at free index 64 of each head.
            # Invalid rows of the last tile (>= vl) are zeroed entirely —
            # this replaces the score mask.
            V = [pers.tile([P, J_list[s], 2, DK + 1], dt_x, name=f"V{s}")
                 for s in range(NB)]
            for s in range(NB):
                J, p0 = J_list[s], P0_list[s]
                if J > 1:
                    nc.vector.memset(V[s][:, 0:J - 1, :, DK], 1.0)
                if p0 < P:
                    # zero whole last tile (partition base stays 0-aligned),
                    # valid rows are overlaid by the ones-memset + V-proj copy
                    nc.vector.memset(V[s][:, J - 1, :, :], 0.0)
                nc.vector.memset(V[s][0:p0, J - 1, :, DK], 1.0)

            AO = [pers.tile([P, T], dt_x, name=f"AO{s}") for s in range(NB)]
            NR = 2 * NT  # unnormalized-output rows per slot (tq, head)
            uo = [pers.tile([DK + 1, NR, 512], BF16, name=f"uo{s}")
                  for s in range(NB)]
            dens = [pers.tile([NR, 512], BF16, name=f"dens{s}")
                    for s in range(NB)]
            recs = [pers.tile([NR, 512], BF16, name=f"rec{s}")
                    for s in range(NB)]

            # ---- emission-unit generators -------------------------------
            def projkv_gen(s):
                J = J_list[s]
                # K projection: weight-stationary per kc over <=4-tile chunks
                ngr = -(-J // 4)
                for g in range(ngr):
                    t0 = 4 * g
                    nt_ = min(4, J - t0)
                    xkw = stream.tile([P, 4, KC, P], dt_x, tag="xk_w")
                    nc.sync.dma_start(xkw[:, 0:nt_], xk_d[s][:, t0:t0 + nt_])
                    ps_k = ps_bg.tile([P, 512], F32, tag="bg")
                    for kc in range(KC):
                        nc.tensor.matmul(ps_k[:, 0:nt_ * P], wk[:, kc, :],
                                         xkw[:, 0:nt_, kc, :],
                                         start=(kc == 0), stop=(kc == KC - 1))
                        if kc == 3:
                            yield
                    nc.vector.tensor_scalar_add(
                        KT[s][:, t0 * P:(t0 + nt_) * P],
                        ps_k[:, 0:nt_ * P], bk[:, 0:1])
                    yield
                # V projection: x-tile stationary, wv moving (N=128)
                p0 = P0_list[s]
                for g in range(ngr):
                    t0 = 4 * g
                    nt_ = min(4, J - t0)
                    xvw = stream.tile([P, 4, KC, P], dt_x, tag="xv_w")
                    nc.sync.dma_start(xvw[:, 0:nt_], xv_d[s][:, t0:t0 + nt_])
                    ps_v = ps_bg.tile([P, 512], F32, tag="bg")
                    for i in range(nt_):
                        reg = ps_v[:, ts(i, P)]
                        for kc in range(KC):
                            nc.tensor.matmul(reg, xvw[:, i, kc, :],
                                             wv[:, kc, :],
                                             start=(kc == 0), stop=False)
                        nc.tensor.matmul(reg, ones_t[0:1, :], bv[0:1, :],
                                         start=False, stop=True)
                        yield
                    for i in range(nt_):
                        t_ = t0 + i
                        rows = p0 if t_ == J - 1 else P
                        nc.vector.tensor_copy(
                            V[s][0:rows, t_, :, 0:DK],
                            ps_v[0:rows, ts(i, P)].rearrange(
                                "p (h d) -> p h d", d=DK))
                    yield

            def projq_gen(s):
                # Q projection: weight-stationary per kc over 4-tile chunks
                for g in range(NT):
                    xqw = stream.tile([P, 4, KC, P], dt_x, tag="xq_w")
                    nc.sync.dma_start(xqw[:], xq_d[s][:, 4 * g:4 * g + 4])
                    ps_q = ps_bg.tile([P, 512], F32, tag="bg")
                    for kc in range(KC):
                        nc.tensor.matmul(ps_q[:], wq[:, kc, :],
                                         xqw[:, :, kc, :],
                                         start=(kc == 0), stop=(kc == KC - 1))
                        if kc == 3:
                            yield
                    nc.vector.tensor_scalar_add(QT[s][:, ts(g, 512)],
                                                ps_q[:], bq[:, 0:1])
                    yield

            def attn_emit(s, pump, uo_act, rate):
                J = J_list[s]
                items = [tuple(range(j, min(j + 2, J)))
                         for j in range(0, J, 2)]
                for tq in range(NT):
                    ps_os = [ps_pv.tile([P, 512], F32, tag="pv",
                                        name=f"pv{h}")
                             for h in range(2)]
                    # unit = (item, head): emit QK+exp for unit u, then the
                    # PV of unit u-1, so the PE never heads-of-line-waits on
                    # an exp (per-engine queues are in-order)
                    pv_pending = deque()
                    for it in items:
                        for h in range(2):
                            pss = ps_qk.tile([P, 2, 512], F32, tag="qk")
                            for k, j_ in enumerate(it):
                                nc.tensor.matmul(
                                    pss[:, k, :],
                                    KT[s][ts(h, DK), ts(j_, P)],
                                    QT[s][ts(h, DK), ts(tq, 512)],
                                    start=True, stop=True,
                                    tile_position=(h * DK, 0))
                            at = attn_pool.tile([P, 2, 512], dt_x, tag="at")
                            if len(it) == 2:
                                nc.scalar.activation(at[:, :, :], pss[:],
                                                     AF.Exp, scale=0.125)
                            else:
                                nc.scalar.activation(at[:, 0, :],
                                                     pss[:, 0, :], AF.Exp,
                                                     scale=0.125)
                            for k, j_ in enumerate(it):
                                pv_pending.append((j_, h, at, k))
                            while len(pv_pending) > 2:
                                _emit_pv(s, tq, ps_os, pv_pending.popleft())
                            pump(rate)
                    while pv_pending:
                        _emit_pv(s, tq, ps_os, pv_pending.popleft())
                    for h in range(2):
                        r = tq * 2 + h
                        # den row rides along at partition DK
                        if uo_act:
                            nc.scalar.activation(uo[s][:, r, :],
                                                 ps_os[h][0:DK + 1, :],
                                                 AF.Identity)
                        else:
                            nc.vector.tensor_copy(uo[s][:, r, :],
                                                  ps_os[h][0:DK + 1, :])
                        nc.sync.dma_start(dens[s][r:r + 1, :],
                                          uo[s][DK:DK + 1, r, :])
                    pump(rate)

            def _emit_pv(s, tq, ps_os, unit):
                J = J_list[s]
                j_, h, at, k = unit
                nc.tensor.matmul(ps_os[h][0:DK + 1, :],
                                 V[s][:, j_, h, :], at[:, k, :],
                                 start=(j_ == 0), stop=(j_ == J - 1))

            def norm_gen(s):
                # batched reciprocal (bf16 is plenty), in 4 pumpable chunks
                with nc.allow_low_precision(reason="bf16 1/den is plenty"):
                    for c in range(4):
                        nc.vector.reciprocal(recs[s][:, ts(c, 128)],
                                             dens[s][:, ts(c, 128)])
                        yield
                for r in range(NR):
                    # stage rec row at partition 0 for the K=1 bcast matmul
                    rst = small.tile([1, 512], BF16, tag="rst")
                    nc.sync.dma_start(rst[:], recs[s][r:r + 1, :])
                    ps_b = ps_bg.tile([P, 512], F32, tag="bg")
                    nc.tensor.matmul(ps_b[0:DK, :], ones_b[0:1, :],
                                     rst[0:1, :], start=True, stop=True)
                    h = r % 2
                    tq = r // 2
                    nc.vector.tensor_mul(
                        out=AO[s][ts(h, DK), ts(tq, 512)],
                        in0=ps_b[0:DK, :], in1=uo[s][0:DK, r, :])
                    yield

            def tail_gen(s):
                # last slot: interleave normalization and out-projection
                # with a one-chunk lag — op units for tq chunk n-1 are
                # emitted between chunk n's norm rows, so the PE reads of
                # AO trail the DVE writes by a full chunk (latency + safety)
                with nc.allow_low_precision(reason="bf16 1/den is plenty"):
                    for c in range(4):
                        nc.vector.reciprocal(recs[s][:, ts(c, 128)],
                                             dens[s][:, ts(c, 128)])
                        yield
                for n in range(NT + 1):
                    if n < NT:
                        for h in range(2):
                            r = 2 * n + h
                            rst = small.tile([1, 512], BF16, tag="rst")
                            nc.sync.dma_start(rst[:], recs[s][r:r + 1, :])
                            ps_b = ps_bg.tile([P, 512], F32, tag="bg")
                            nc.tensor.matmul(ps_b[0:DK, :], ones_b[0:1, :],
                                             rst[0:1, :],
                                             start=True, stop=True)
                            nc.vector.tensor_mul(
                                out=AO[s][ts(h, DK), ts(n, 512)],
                                in0=ps_b[0:DK, :], in1=uo[s][0:DK, r, :])
                            yield
                    if n >= 1:
                        m = n - 1
                        for dt_i in range(KC):
                            ps_op = ps_bg.tile([P, 512], F32, tag="bg")
                            nc.tensor.matmul(ps_op[:], wo[:, dt_i, :],
                                             AO[s][:, ts(m, 512)],
                                             start=True, stop=True)
                            ot = outp.tile([P, 512], BF16, tag="ot2")
                            if dt_i % 2 == 0:
                                nc.scalar.activation(ot[:], ps_op[:],
                                                     AF.Identity)
                            else:
                                nc.vector.tensor_copy(ot[:], ps_op[:])
                            nc.sync.dma_start(
                                o_d[s][dt_i][:, ts(m, 512)], ot[:])
                            yield

            def outproj_gen(s, act_mode):
                # output projection; 4 n-chunk copies staged into one
                # [128, 2048] tile -> ONE DMA per dt row.
                # act_mode: 0 = DVE-only copies (exp-saturated window),
                #           1 = alternate ACT/DVE (exp-sparse / tail)
                unit = 0
                for dt_i in range(KC):
                    ot = outp.tile([P, NT, 512], BF16, tag="ot")
                    for n in range(NT):
                        ps_op = ps_bg.tile([P, 512], F32, tag="bg")
                        nc.tensor.matmul(ps_op[:], wo[:, dt_i, :],
                                         AO[s][:, ts(n, 512)],
                                         start=True, stop=True)
                        if act_mode and unit % 2 == 0:
                            nc.scalar.activation(ot[:, n, :], ps_op[:],
                                                 AF.Identity)
                        else:
                            nc.vector.tensor_copy(ot[:, n, :], ps_op[:])
                        unit += 1
                        yield
                    nc.sync.dma_start(o_d[s][dt_i], ot[:])
                    yield

            # ---- software-pipelined emission ----------------------------
            # entries: (kind, slot, generator). proj entries are
            # DEADLINE-BOUND (slot s's projections must be fully emitted
            # before attn_emit(s)); norm/op/tail are background.
            pending = deque()

            def _advance(want_proj, n):
                k = 0
                for ent in list(pending):
                    if (ent[0] == "proj") != want_proj:
                        continue
                    while k < n:
                        try:
                            next(ent[2])
                            k += 1
                        except StopIteration:
                            pending.remove(ent)
                            break
                    break
                return k

            def pump(n=2):
                # give projections half the budget (they gate the next
                # slot's attention), background the rest
                used = _advance(True, max(1, n // 2))
                _advance(False, max(1, n - used))

            def drain_projs():
                # finish ALL remaining proj entries (wherever they sit in
                # the deque), keeping DVE/ACT fed with background units
                while any(e[0] == "proj" for e in pending):
                    _advance(True, 2)
                    _advance(False, 1)

            # slot 0 projections run serially up front; the remaining
            # persistent DMAs are emitted between phases so the trigger
            # queue serves the first x tiles ASAP
            for u in projkv_gen(0):
                pass
            nc.sync.dma_start(wq[:], wq_d[:])
            nc.sync.dma_start(bq[:], bq_d[:])
            for u in projq_gen(0):
                pass
            nc.sync.dma_start(wo[:], wo_d[:])
            nc.sync.dma_start(ones_b[:], onesb_d[:])

            for s in range(NB):
                if s + 1 < NB:
                    pending.append(("proj", s + 1, projkv_gen(s + 1)))
                    pending.append(("proj", s + 1, projq_gen(s + 1)))
                if s >= 1:
                    pending.append(("norm", s - 1, norm_gen(s - 1)))
                    # big-J window is exp-saturated: DVE-only copies there
                    pending.append(("op", s - 1, outproj_gen(s - 1,
                                                             J_list[s] < 8)))
                assert not any(e[0] == "proj" and e[1] == s
                               for e in pending), \
                    f"slot {s} attention emitted before its projections"
                attn_emit(s, pump, uo_act=(J_list[s] < 8),
                          rate=3 if J_list[s] >= 8 else 2)
                drain_projs()
            pending.append(("tail", NB - 1, tail_gen(NB - 1)))
            while pending:
                pump(1000)

    _split_multi_waits(nc)
    return nc


_CACHE = {}


def _get_nc(NB, J_list, P0_list, dt_x):
    key = (NB, tuple(J_list), tuple(P0_list), str(dt_x))
    if key not in _CACHE:
        _CACHE[key] = build_nc(NB, J_list, P0_list, dt_x)
    return _CACHE[key]


def _xt(x, dt_np, ntiles=TC):
    """[T, D] -> [P, ntiles, KC, 128] partition-major tile layout."""
    xt = x.T.reshape(KC, P, TC, P).transpose(1, 2, 0, 3)[:, :ntiles]
    return np.ascontiguousarray(xt).astype(dt_np)


def kernel(**inputs):
    query = np.asarray(inputs["query"], dtype=np.float32)
    key = np.asarray(inputs["key"], dtype=np.float32)
    value = np.asarray(inputs["value"], dtype=np.float32)
    vl = np.asarray(inputs["valid_length"]).astype(np.int64)
    W_q = np.asarray(inputs["W_q"], dtype=np.float32)
    b_q = np.asarray(inputs["b_q"], dtype=np.float32)
    W_k = np.asarray(inputs["W_k"], dtype=np.float32)
    b_k = np.asarray(inputs["b_k"], dtype=np.float32)
    W_v = np.asarray(inputs["W_v"], dtype=np.float32)
    b_v = np.asarray(inputs["b_v"], dtype=np.float32)
    W_o = np.asarray(inputs["W_o"], dtype=np.float32)
    b_o = np.asarray(inputs["b_o"], dtype=np.float32)

    B = query.shape[0]
    NB = B
    CPB = (H // NCORES) * DK       # 2 heads per core -> 128 cols
    dt_x = BF16
    dt_np = BF16_NP

    # slot order: descending J, but the LARGEST batch goes LAST — its long
    # exp window absorbs the other slots' out-projections and norms
    Jv = np.where(vl == 0, TC * P, np.minimum(vl, TC * P))
    desc = list(np.argsort(-Jv, kind="stable"))
    order = desc[1:] + desc[:1]
    J_list = []
    P0_list = []
    for s in range(NB):
        v = int(vl[order[s]])
        if v == 0 or v >= T:
            J_list.append(TC)
            P0_list.append(P)
        else:
            J = max(1, -(-v // P))
            J_list.append(J)
            p0 = v - (J - 1) * P
            P0_list.append(p0)

    nc = _get_nc(NB, J_list, P0_list, dt_x)

    # host-side shard prep
    xq_np, xk_np, xv_np = [], [], []
    for s in range(NB):
        b = int(order[s])
        v = int(vl[b])
        J = J_list[s]
        q_b = query[b] if v != 0 else np.zeros_like(query[b])
        xq_np.append(_xt(q_b, dt_np))
        xk_np.append(_xt(key[b], dt_np, J))
        xv_np.append(_xt(value[b], dt_np, J))

    in_maps = []
    for c in range(NCORES):
        c0 = c * CPB
        cols = slice(c0, c0 + CPB)
        im = {
            "wq": np.ascontiguousarray(
                W_q.reshape(KC, P, H * DK).transpose(1, 0, 2)[:, :, cols]
            ).astype(dt_np),
            "wk": np.ascontiguousarray(
                W_k.reshape(KC, P, H * DK).transpose(1, 0, 2)[:, :, cols]
            ).astype(dt_np),
            "wv": np.ascontiguousarray(
                W_v.reshape(KC, P, H * DK).transpose(1, 0, 2)[:, :, cols]
            ).astype(dt_np),
            "wo": np.ascontiguousarray(
                W_o[cols].reshape(P, KC, P)).astype(dt_np),
            "bq": np.ascontiguousarray(b_q[cols][:, None]).astype(np.float32),
            "bk": np.ascontiguousarray(b_k[cols][:, None]).astype(np.float32),
            "bv": np.ascontiguousarray(b_v[cols][None, :]).astype(dt_np),
        }
        im["onesb"] = np.ones((1, DK), BF16_NP)
        for s in range(NB):
            im[f"xq{s}"] = xq_np[s]
            im[f"xk{s}"] = xk_np[s]
            im[f"xv{s}"] = xv_np[s]
        in_maps.append(im)

    res = run_bass_kernel_spmd(nc, in_maps, list(range(NCORES)))

    out = np.zeros((B, T, D), np.float32)
    for s in range(NB):
        b = int(order[s])
        acc = np.zeros((D, T), np.float32)
        for c in range(NCORES):
            acc += np.asarray(res.results[c][f"o{s}"]
                              ).astype(np.float32).reshape(D, T)
        out[b] = acc.T + b_o[None, :]
    return out


# revision 21
# speedup vs baseline: 1.2345x; 1.0460x over previous
"""Trainium2 Bass kernel for nn_MultiHeadAttention (B=4, T=2048, D=1024,
H=16, d_k=64) on 8 NeuronCores.

Sharding: tensor-parallel over heads — core c computes heads {2c, 2c+1} for
ALL batches (W_q/W_k/W_v column-sharded, W_o row-sharded). The final
all-reduce of the output projection is replaced by a host-side sum of the 8
partial outputs (each written as [KC, 128, T]).

Design (v10 — rebuilt from the v9 trace):
  - K/V are projected (and their x DMA'd) only for the ceil(vl/128) Tk
    tiles that attention actually reads.
  - scores^T layout (Tk on partitions, Tq free): the two heads' QK^T
    matmuls are K=64 row-tile pairs sharing the PE array concurrently via
    tile_position; two Tk tiles merge into one [128, 2x512] ACT exp.
  - NO score masking: invalid key rows (>= vl, last tile only) are instead
    zeroed in V (both the value columns and the folded ones-column), so
    their exp'd scores contribute nothing to numerator or denominator.
    Removes 32 K=1 premix matmuls + the mask DMAs.
  - softmax denominator rides as a ones-column folded into the P@V matmul
    (lhsT = [V_h | 1]); un-normalized outputs + den rows staged per tq;
    dens gathered with ONE DMA per slot, reciprocal'd in 4 pumpable DVE
    chunks, re-staged to partition 0 with ONE DMA, broadcast by K=1
    matmuls, applied by DVE multiplies.
  - slot order: descending J with the LARGEST batch LAST — its long exp
    window (ACT-bound, PE slack) absorbs the out-projections and
    normalizations of all earlier slots, which previously formed a
    ~150us copy-bound serial tail with a cold (HAM-throttled) PE.
  - out-projection stages 4 n-chunks into a [128, 2048] SBUF tile and
    writes ONE DMA per (slot, dt): 32 output DMAs instead of 128 (the
    Sync trigger queue at ~707ns/DMA was 70%+ busy in the tail).
  - PSUM->SBUF copies are steered: ACT (Identity) in exp-sparse windows,
    DVE in the exp-saturated big window, alternating in drain/tail.
  - emission is software-pipelined at instruction granularity (in-order
    per-engine queues): proj/norm/outproj generators are pumped between
    attention units; the pre-big-slot drain interleaves proj with
    norm/outproj so DVE/ACT stay fed while the PE runs projections.
  - trn2 encodes at most one semaphore wait per instruction; a post-pass
    splits any multi-wait instruction Tile emits.
"""
import os
import sys
from collections import deque

for _p in ("/opt/trn_rl_repo", "/root/.axon_site/_ro/trn_rl_repo"):
    if os.path.isdir(_p) and _p not in sys.path:
        sys.path.append(_p)

import numpy as np
import ml_dtypes

import concourse.bass as bass
import concourse.mybir as mybir
import concourse.tile as tile
from concourse.bass import ts
from concourse.bass_utils import run_bass_kernel_spmd

D = 1024
T = 2048
H = 16
DK = 64
P = 128
KC = D // P          # 8 contraction chunks for the projections
TC = T // P          # 16 token tiles of 128
NT = T // 512        # 4 Tq chunks of 512
NCORES = 8

F32 = mybir.dt.float32
BF16 = mybir.dt.bfloat16
AF = mybir.ActivationFunctionType
BF16_NP = ml_dtypes.bfloat16


def _split_multi_waits(nc):
    """trn2 instructions encode at most one sync wait; split the rest into
    standalone single-wait event-semaphore ops."""
    n_split = 0
    for f in nc.m.functions:
        for blk in f.blocks:
            insts = blk.instructions
            out = []
            changed = False
            for inst in insts:
                si = inst.sync_info
                if si is not None and len(si.on_wait) > 1:
                    waits = list(si.on_wait)
                    for k, wt in enumerate(waits[:-1]):
                        ev = mybir.InstEventSemaphore(
                            name=f"{inst.name}_wsplit{k}",
                            engine=inst.engine,
                            ins=[],
                            outs=[],
                            bass_nofuse=True,
                            sync_info=mybir.SyncInfo(on_wait=[wt], on_update=[]),
                        )
                        out.append(ev)
                        n_split += 1
                    inst.sync_info = mybir.SyncInfo(
                        on_wait=[waits[-1]], on_update=si.on_update
                    )
                    changed = True
                out.append(inst)
            if changed:
                blk.instructions = out
    return n_split


def build_nc(NB, J_list, P0_list, dt_x):
    """Build the SPMD program.

    NB      : number of batch slots handled per core
    J_list  : per batch slot, number of 128-row Tk tiles of attention
    P0_list : per batch slot, valid rows in the LAST Tk tile (1..128)
    dt_x    : dtype of x/weights/intermediates
    """
    nc = bass.Bass()

    # partition-major tile layout: per partition, each 128-token tile is a
    # contiguous [KC, 128] run (2 KB bf16)
    xq_d = [nc.declare_dram_parameter(f"xq{s}", [P, TC, KC, P], dt_x,
                                      isOutput=False) for s in range(NB)]
    xk_d = [nc.declare_dram_parameter(f"xk{s}", [P, J_list[s], KC, P], dt_x,
                                      isOutput=False) for s in range(NB)]
    xv_d = [nc.declare_dram_parameter(f"xv{s}", [P, J_list[s], KC, P], dt_x,
                                      isOutput=False) for s in range(NB)]
    wq_d = nc.declare_dram_parameter("wq", [P, KC, P], dt_x, isOutput=False)
    wk_d = nc.declare_dram_parameter("wk", [P, KC, P], dt_x, isOutput=False)
    wv_d = nc.declare_dram_parameter("wv", [P, KC, P], dt_x, isOutput=False)
    wo_d = nc.declare_dram_parameter("wo", [P, KC, P], dt_x, isOutput=False)
    bq_d = nc.declare_dram_parameter("bq", [P, 1], F32, isOutput=False)
    bk_d = nc.declare_dram_parameter("bk", [P, 1], F32, isOutput=False)
    bv_d = nc.declare_dram_parameter("bv", [1, P], dt_x, isOutput=False)
    onesb_d = nc.declare_dram_parameter("onesb", [1, DK], BF16,
                                        isOutput=False)
    o_d = [nc.declare_dram_parameter(f"o{s}", [KC, P, T], BF16, isOutput=True)
           for s in range(NB)]

    with tile.TileContext(nc) as tc:
        with (
            tc.tile_pool(name="pers", bufs=1) as pers,
            tc.tile_pool(name="stream", bufs=3) as stream,
            tc.tile_pool(name="attn", bufs=4) as attn_pool,
            tc.tile_pool(name="small", bufs=2) as small,
            tc.tile_pool(name="outp", bufs=3) as outp,
            tc.tile_pool(name="ps_qk", bufs=2, space="PSUM") as ps_qk,
            tc.tile_pool(name="ps_pv", bufs=2, space="PSUM") as ps_pv,
            tc.tile_pool(name="ps_bg", bufs=2, space="PSUM") as ps_bg,
        ):
            # ---- persistent tensors -------------------------------------
            wq = pers.tile([P, KC, P], dt_x, name="wq")
            wk = pers.tile([P, KC, P], dt_x, name="wk")
            wv = pers.tile([P, KC, P], dt_x, name="wv")
            wo = pers.tile([P, KC, P], dt_x, name="wo")
            bq = pers.tile([P, 1], F32, name="bq")
            bk = pers.tile([P, 1], F32, name="bk")
            bv = pers.tile([1, P], dt_x, name="bv")
            # HAM warm-up: dep-free junk matmuls keep the PE busy while the
            # first x tiles stream in, so real projections start at 2.4 GHz
            # (the PE clock-gate needs ~3.4us of sustained activity).
            # Results land in a scratch psum bank and are never read.
            warm_sb = pers.tile([P, 512], dt_x, name="warm_sb")
            nc.vector.memset(warm_sb[:], 1.0)
            ps_warm = ps_bg.tile([P, 512], F32, tag="bg")
            for _ in range(30):
                nc.tensor.matmul(ps_warm[:], warm_sb[:, 0:128], warm_sb[:],
                                 start=True, stop=True)
            # front-load only what the first K projection needs; remaining
            # weights are emitted between phases
            nc.sync.dma_start(wk[:], wk_d[:])
            xkw0 = stream.tile([P, 4, KC, P], dt_x, tag="xk_w")
            nt0 = min(4, J_list[0])
            nc.sync.dma_start(xkw0[:, 0:nt0], xk_d[0][:, 0:nt0])
            nc.sync.dma_start(wv[:], wv_d[:])
            nc.sync.dma_start(bk[:], bk_d[:])
            nc.sync.dma_start(bv[:], bv_d[:])

            ones_t = pers.tile([1, P], dt_x, name="ones_t")   # V-bias fold lhsT
            nc.vector.memset(ones_t[:], 1.0)
            ones_b = pers.tile([1, DK], BF16, name="ones_b")  # 1/den bcast lhsT

            QT = [pers.tile([P, T], dt_x, name=f"QT{s}") for s in range(NB)]
            KT = [pers.tile([P, J_list[s] * P], dt_x, name=f"KT{s}")
                  for s in range(NB)]
            # V with a ones column folded in at free index 64 of each head.
            # Invalid rows of the last tile (>= vl) are zeroed entirely —
            # this replaces the score mask.
            V = [pers.tile([P, J_list[s], 2, DK + 1], dt_x, name=f"V{s}")
                 for s in range(NB)]
            for s in range(NB):
                J, p0 = J_list[s], P0_list[s]
                if J > 1:
                    nc.vector.memset(V[s][:, 0:J - 1, :, DK], 1.0)
                if p0 < P:
                    # zero whole last tile (partition base stays 0-aligned),
                    # valid rows are overlaid by the ones-memset + V-proj copy
                    nc.vector.memset(V[s][:, J - 1, :, :], 0.0)
                nc.vector.memset(V[s][0:p0, J - 1, :, DK], 1.0)

            AO = [pers.tile([P, T], dt_x, name=f"AO{s}") for s in range(NB)]
            NR = 2 * NT  # unnormalized-output rows per slot (tq, head)
            uo = [pers.tile([DK + 1, NR, 512], BF16, name=f"uo{s}")
                  for s in range(NB)]
            dens = [pers.tile([NR, 512], BF16, name=f"dens{s}")
                    for s in range(NB)]
            recs = [pers.tile([NR, 512], BF16, name=f"rec{s}")
                    for s in range(NB)]
            # last slot streams its normalization per tq; engine APs need
            # partition base 0, so its den/rec rows live in per-tq tiles
            densq = [pers.tile([2, 512], BF16, name=f"densq{q}")
                     for q in range(NT)]
            recsq = [pers.tile([2, 512], BF16, name=f"recsq{q}")
                     for q in range(NT)]

            # ---- emission-unit generators -------------------------------
            def projkv_gen(s, xkw_pre=None):
                J = J_list[s]
                # K projection: weight-stationary per kc over <=4-tile chunks
                ngr = -(-J // 4)
                for g in range(ngr):
                    t0 = 4 * g
                    nt_ = min(4, J - t0)
                    if g == 0 and xkw_pre is not None:
                        xkw = xkw_pre
                    else:
                        xkw = stream.tile([P, 4, KC, P], dt_x, tag="xk_w")
                        nc.sync.dma_start(xkw[:, 0:nt_],
                                          xk_d[s][:, t0:t0 + nt_])
                    ps_k = ps_bg.tile([P, 512], F32, tag="bg")
                    for kc in range(KC):
                        nc.tensor.matmul(ps_k[:, 0:nt_ * P], wk[:, kc, :],
                                         xkw[:, 0:nt_, kc, :],
                                         start=(kc == 0), stop=(kc == KC - 1))
                        if kc == 3:
                            yield
                    nc.vector.tensor_scalar_add(
                        KT[s][:, t0 * P:(t0 + nt_) * P],
                        ps_k[:, 0:nt_ * P], bk[:, 0:1])
                    yield
                # V projection: x-tile stationary, wv moving (N=128)
                p0 = P0_list[s]
                for g in range(ngr):
                    t0 = 4 * g
                    nt_ = min(4, J - t0)
                    xvw = stream.tile([P, 4, KC, P], dt_x, tag="xv_w")
                    nc.sync.dma_start(xvw[:, 0:nt_], xv_d[s][:, t0:t0 + nt_])
                    ps_v = ps_bg.tile([P, 512], F32, tag="bg")
                    for i in range(nt_):
                        reg = ps_v[:, ts(i, P)]
                        for kc in range(KC):
                            nc.tensor.matmul(reg, xvw[:, i, kc, :],
                                             wv[:, kc, :],
                                             start=(kc == 0), stop=False)
                        nc.tensor.matmul(reg, ones_t[0:1, :], bv[0:1, :],
                                         start=False, stop=True)
                        yield
                    for i in range(nt_):
                        t_ = t0 + i
                        rows = p0 if t_ == J - 1 else P
                        nc.vector.tensor_copy(
                            V[s][0:rows, t_, :, 0:DK],
                            ps_v[0:rows, ts(i, P)].rearrange(
                                "p (h d) -> p h d", d=DK))
                    yield

            def projq_gen(s):
                # Q projection: weight-stationary per kc over 4-tile chunks
                for g in range(NT):
                    xqw = stream.tile([P, 4, KC, P], dt_x, tag="xq_w")
                    nc.sync.dma_start(xqw[:], xq_d[s][:, 4 * g:4 * g + 4])
                    ps_q = ps_bg.tile([P, 512], F32, tag="bg")
                    for kc in range(KC):
                        nc.tensor.matmul(ps_q[:], wq[:, kc, :],
                                         xqw[:, :, kc, :],
                                         start=(kc == 0), stop=(kc == KC - 1))
                        if kc == 3:
                            yield
                    nc.vector.tensor_scalar_add(QT[s][:, ts(g, 512)],
                                                ps_q[:], bq[:, 0:1])
                    yield

            def attn_emit(s, pump, uo_act, rate, tail_factory=None):
                J = J_list[s]
                items = [tuple(range(j, min(j + 2, J)))
                         for j in range(0, J, 2)]
                for tq in range(NT):
                    ps_os = [ps_pv.tile([P, 512], F32, tag="pv",
                                        name=f"pv{h}")
                             for h in range(2)]
                    # unit = (item, head): emit QK+exp for unit u, then the
                    # PV of unit u-1, so the PE never heads-of-line-waits on
                    # an exp (per-engine queues are in-order)
                    pv_pending = deque()
                    for it in items:
                        for h in range(2):
                            pss = ps_qk.tile([P, 2, 512], F32, tag="qk")
                            for k, j_ in enumerate(it):
                                nc.tensor.matmul(
                                    pss[:, k, :],
                                    KT[s][ts(h, DK), ts(j_, P)],
                                    QT[s][ts(h, DK), ts(tq, 512)],
                                    start=True, stop=True,
                                    tile_position=(h * DK, 0))
                            at = attn_pool.tile([P, 2, 512], dt_x, tag="at")
                            if len(it) == 2:
                                nc.scalar.activation(at[:, :, :], pss[:],
                                                     AF.Exp, scale=0.125)
                            else:
                                nc.scalar.activation(at[:, 0, :],
                                                     pss[:, 0, :], AF.Exp,
                                                     scale=0.125)
                            for k, j_ in enumerate(it):
                                pv_pending.append((j_, h, at, k))
                            while len(pv_pending) > 2:
                                _emit_pv(s, tq, ps_os, pv_pending.popleft())
                            pump(rate)
                    while pv_pending:
                        _emit_pv(s, tq, ps_os, pv_pending.popleft())
                    for h in range(2):
                        r = tq * 2 + h
                        # den row rides along at partition DK
                        if uo_act:
                            nc.scalar.activation(uo[s][:, r, :],
                                                 ps_os[h][0:DK + 1, :],
                                                 AF.Identity)
                        else:
                            nc.vector.tensor_copy(uo[s][:, r, :],
                                                  ps_os[h][0:DK + 1, :])
                        if tail_factory is None:
                            nc.sync.dma_start(dens[s][r:r + 1, :],
                                              uo[s][DK:DK + 1, r, :])
                        else:
                            nc.sync.dma_start(densq[tq][h:h + 1, :],
                                              uo[s][DK:DK + 1, r, :])
                    if tail_factory is not None:
                        pending.append(("tail", s, tail_factory(tq)))
                    pump(rate)

            def _emit_pv(s, tq, ps_os, unit):
                J = J_list[s]
                j_, h, at, k = unit
                nc.tensor.matmul(ps_os[h][0:DK + 1, :],
                                 V[s][:, j_, h, :], at[:, k, :],
                                 start=(j_ == 0), stop=(j_ == J - 1))

            def norm_gen(s):
                # batched reciprocal (bf16 is plenty), in 4 pumpable chunks
                with nc.allow_low_precision(reason="bf16 1/den is plenty"):
                    for c in range(4):
                        nc.vector.reciprocal(recs[s][:, ts(c, 128)],
                                             dens[s][:, ts(c, 128)])
                        yield
                for r in range(NR):
                    # stage rec row at partition 0 for the K=1 bcast matmul
                    rst = small.tile([1, 512], BF16, tag="rst")
                    nc.sync.dma_start(rst[:], recs[s][r:r + 1, :])
                    ps_b = ps_bg.tile([P, 512], F32, tag="bg")
                    nc.tensor.matmul(ps_b[0:DK, :], ones_b[0:1, :],
                                     rst[0:1, :], start=True, stop=True)
                    h = r % 2
                    tq = r // 2
                    nc.vector.tensor_mul(
                        out=AO[s][ts(h, DK), ts(tq, 512)],
                        in0=ps_b[0:DK, :], in1=uo[s][0:DK, r, :])
                    yield

            def tail_tq_gen(s, tq):
                # streamed last-slot normalization + out-projection for one
                # tq chunk, pumped inside the remaining attention window.
                # recip is lane-starved at 2 rows, so chunk its columns.
                with nc.allow_low_precision(reason="bf16 1/den is plenty"):
                    for c in range(4):
                        nc.vector.reciprocal(recsq[tq][:, ts(c, 128)],
                                             densq[tq][:, ts(c, 128)])
                        yield
                for h in range(2):
                    r = 2 * tq + h
                    rst = small.tile([1, 512], BF16, tag="rst")
                    nc.sync.dma_start(rst[:], recsq[tq][h:h + 1, :])
                    ps_b = ps_bg.tile([P, 512], F32, tag="bg")
                    nc.tensor.matmul(ps_b[0:DK, :], ones_b[0:1, :],
                                     rst[0:1, :], start=True, stop=True)
                    nc.vector.tensor_mul(
                        out=AO[s][ts(h, DK), ts(tq, 512)],
                        in0=ps_b[0:DK, :], in1=uo[s][0:DK, r, :])
                    yield
                for dt_i in range(KC):
                    ps_op = ps_bg.tile([P, 512], F32, tag="bg")
                    nc.tensor.matmul(ps_op[:], wo[:, dt_i, :],
                                     AO[s][:, ts(tq, 512)],
                                     start=True, stop=True)
                    ot = outp.tile([P, 512], BF16, tag="ot2")
                    if dt_i % 2 == 0:
                        nc.scalar.activation(ot[:], ps_op[:], AF.Identity)
                    else:
                        nc.vector.tensor_copy(ot[:], ps_op[:])
                    nc.sync.dma_start(o_d[s][dt_i][:, ts(tq, 512)], ot[:])
                    yield

            def outproj_gen(s, act_mode):
                # output projection; 4 n-chunk copies staged into one
                # [128, 2048] tile -> ONE DMA per dt row.
                # act_mode: 0 = DVE-only copies (exp-saturated window),
                #           1 = alternate ACT/DVE (exp-sparse / tail)
                unit = 0
                for dt_i in range(KC):
                    ot = outp.tile([P, NT, 512], BF16, tag="ot")
                    for n in range(NT):
                        ps_op = ps_bg.tile([P, 512], F32, tag="bg")
                        nc.tensor.matmul(ps_op[:], wo[:, dt_i, :],
                                         AO[s][:, ts(n, 512)],
                                         start=True, stop=True)
                        if act_mode and unit % 2 == 0:
                            nc.scalar.activation(ot[:, n, :], ps_op[:],
                                                 AF.Identity)
                        else:
                            nc.vector.tensor_copy(ot[:, n, :], ps_op[:])
                        unit += 1
                        yield
                    nc.sync.dma_start(o_d[s][dt_i], ot[:])
                    yield

            # ---- software-pipelined emission ----------------------------
            # entries: (kind, slot, generator). proj entries are
            # DEADLINE-BOUND (slot s's projections must be fully emitted
            # before attn_emit(s)); norm/op/tail are background.
            pending = deque()

            def _advance(want_proj, n):
                k = 0
                for ent in list(pending):
                    if (ent[0] == "proj") != want_proj:
                        continue
                    while k < n:
                        try:
                            next(ent[2])
                            k += 1
                        except StopIteration:
                            pending.remove(ent)
                            break
                    break
                return k

            def pump(n=2):
                # give projections half the budget (they gate the next
                # slot's attention), background the rest
                used = _advance(True, max(1, n // 2))
                _advance(False, max(1, n - used))

            def drain_projs():
                # finish ALL remaining proj entries (wherever they sit in
                # the deque), keeping DVE/ACT fed with background units
                while any(e[0] == "proj" for e in pending):
                    _advance(True, 2)
                    _advance(False, 1)

            # slot 0 projections run serially up front; the remaining
            # persistent DMAs are emitted between phases so the trigger
            # queue serves the first x tiles ASAP
            for u in projkv_gen(0, xkw_pre=xkw0):
                pass
            nc.sync.dma_start(wq[:], wq_d[:])
            nc.sync.dma_start(bq[:], bq_d[:])
            for u in projq_gen(0):
                pass
            nc.sync.dma_start(wo[:], wo_d[:])
            nc.sync.dma_start(ones_b[:], onesb_d[:])

            for s in range(NB):
                if s + 1 < NB:
                    pending.append(("proj", s + 1, projkv_gen(s + 1)))
                    pending.append(("proj", s + 1, projq_gen(s + 1)))
                if s >= 1:
                    pending.append(("norm", s - 1, norm_gen(s - 1)))
                    pending.append(("op", s - 1, outproj_gen(s - 1, True)))
                assert not any(e[0] == "proj" and e[1] == s
                               for e in pending), \
                    f"slot {s} attention emitted before its projections"
                last = s == NB - 1
                attn_emit(s, pump, uo_act=(J_list[s] < 8),
                          rate=4 if J_list[s] >= 8 else 2,
                          tail_factory=(
                              (lambda tq: tail_tq_gen(s, tq)) if last
                              else None))
                drain_projs()
            while pending:
                pump(1000)

    _split_multi_waits(nc)
    return nc


_CACHE = {}


def _get_nc(NB, J_list, P0_list, dt_x):
    key = (NB, tuple(J_list), tuple(P0_list), str(dt_x))
    if key not in _CACHE:
        _CACHE[key] = build_nc(NB, J_list, P0_list, dt_x)
    return _CACHE[key]


def _xt(x, dt_np, ntiles=TC):
    """[T, D] -> [P, ntiles, KC, 128] partition-major tile layout."""
    xt = x.T.reshape(KC, P, TC, P).transpose(1, 2, 0, 3)[:, :ntiles]
    return np.ascontiguousarray(xt).astype(dt_np)


def kernel(**inputs):
    query = np.asarray(inputs["query"], dtype=np.float32)
    key = np.asarray(inputs["key"], dtype=np.float32)
    value = np.asarray(inputs["value"], dtype=np.float32)
    vl = np.asarray(inputs["valid_length"]).astype(np.int64)
    W_q = np.asarray(inputs["W_q"], dtype=np.float32)
    b_q = np.asarray(inputs["b_q"], dtype=np.float32)
    W_k = np.asarray(inputs["W_k"], dtype=np.float32)
    b_k = np.asarray(inputs["b_k"], dtype=np.float32)
    W_v = np.asarray(inputs["W_v"], dtype=np.float32)
    b_v = np.asarray(inputs["b_v"], dtype=np.float32)
    W_o = np.asarray(inputs["W_o"], dtype=np.float32)
    b_o = np.asarray(inputs["b_o"], dtype=np.float32)

    B = query.shape[0]
    NB = B
    CPB = (H // NCORES) * DK       # 2 heads per core -> 128 cols
    dt_x = BF16
    dt_np = BF16_NP

    # slot order: descending J, but the LARGEST batch goes LAST — its long
    # exp window absorbs the other slots' out-projections and norms
    Jv = np.where(vl == 0, TC * P, np.minimum(vl, TC * P))
    desc = list(np.argsort(-Jv, kind="stable"))
    order = desc[1:] + desc[:1]
    J_list = []
    P0_list = []
    for s in range(NB):
        v = int(vl[order[s]])
        if v == 0 or v >= T:
            J_list.append(TC)
            P0_list.append(P)
        else:
            J = max(1, -(-v // P))
            J_list.append(J)
            p0 = v - (J - 1) * P
            P0_list.append(p0)

    nc = _get_nc(NB, J_list, P0_list, dt_x)

    # host-side shard prep
    xq_np, xk_np, xv_np = [], [], []
    for s in range(NB):
        b = int(order[s])
        v = int(vl[b])
        J = J_list[s]
        q_b = query[b] if v != 0 else np.zeros_like(query[b])
        xq_np.append(_xt(q_b, dt_np))
        xk_np.append(_xt(key[b], dt_np, J))
        xv_np.append(_xt(value[b], dt_np, J))

    in_maps = []
    for c in range(NCORES):
        c0 = c * CPB
        cols = slice(c0, c0 + CPB)
        im = {
            "wq": np.ascontiguousarray(
                W_q.reshape(KC, P, H * DK).transpose(1, 0, 2)[:, :, cols]
            ).astype(dt_np),
            "wk": np.ascontiguousarray(
                W_k.reshape(KC, P, H * DK).transpose(1, 0, 2)[:, :, cols]
            ).astype(dt_np),
            "wv": np.ascontiguousarray(
                W_v.reshape(KC, P, H * DK).transpose(1, 0, 2)[:, :, cols]
            ).astype(dt_np),
            "wo": np.ascontiguousarray(
                W_o[cols].reshape(P, KC, P)).astype(dt_np),
            "bq": np.ascontiguousarray(b_q[cols][:, None]).astype(np.float32),
            "bk": np.ascontiguousarray(b_k[cols][:, None]).astype(np.float32),
            "bv": np.ascontiguousarray(b_v[cols][None, :]).astype(dt_np),
        }
        im["onesb"] = np.ones((1, DK), BF16_NP)
        for s in range(NB):
            im[f"xq{s}"] = xq_np[s]
            im[f"xk{s}"] = xk_np[s]
            im[f"xv{s}"] = xv_np[s]
        in_maps.append(im)

    res = run_bass_kernel_spmd(nc, in_maps, list(range(NCORES)))

    out = np.zeros((B, T, D), np.float32)
    for s in range(NB):
        b = int(order[s])
        acc = np.zeros((D, T), np.float32)
        for c in range(NCORES):
            acc += np.asarray(res.results[c][f"o{s}"]
                              ).astype(np.float32).reshape(D, T)
        out[b] = acc.T + b_o[None, :]
    return out
